# revision 19
# baseline (speedup 1.0000x reference)
"""GCN (gather/segment-sum message passing) + mean-pool + MLP on 8 TRN2 cores.

Single-launch design (data-parallel over graphs per the sharding hint):
 - nodes/graphs sharded contiguously across 8 cores (batch is sorted); every
   edge is owned by the core owning its TARGET (col) node.
 - Phase 1 (on device, every core, replicated): y = rsqrt(deg)*(x @ W_gcn)
   for ALL nodes via bf16 matmuls; y rows are written to an HBM table in
   RANK order r(n) = (n%128)*784 + n//128 so each [128, G, 64] SBUF tile
   dumps with one contiguous 256B*G descriptor per partition.  Ranks of the
   zero-padded x columns double as gather zero rows.  Banks for int16
   gather indices are rank quarters (= partition groups of 32).
 - Phase 2: per source-bank, edges are organized into "prefix rounds"
   (nodes sorted by per-bank in-degree; round r gathers the r-th in-edge of
   every node that has one), so each dma_gather chunk is POSITION-ALIGNED
   with the accumulator and aggregation is plain DVE adds (round 0 is a
   copy; only the never-touched accumulator tail is memset).  Gathers run
   on 4 SWDGE queues round-robin.
 - Phase 3: bank partials merged by permute-gather through an HBM scratch
   (bank 0 = copy, no z memset).
 - Phase 4: z = relu(dinv*z + b), graph mean-pool via one-hot PSUM
   matmuls, 64->64->2 MLP + sigmoid.  Output (64,2) per core, concat on
   host -> (512,2).
"""

import os
import sys

sys.path.insert(0, "/opt/trn_rl_repo")

import numpy as np

import concourse.bacc as bacc
import concourse.bass as bass
import concourse.mybir as mybir
import concourse.tile as tile
from concourse.bass_utils import run_bass_kernel_spmd

NC = 8            # cores
NB = 4            # source banks (int16 gather index limit)
NQ = 4            # SWDGE queues, round-robin over gather chunks
P = 128
HID = 64
NT = 784          # node blocks of 128 (100352 rank slots >= 100000), 4*196
VB = NT * 32 * 1  # bank size in rank space: 32 partitions * 784 = 25088
DUMMY = 783       # bank-local rank of a guaranteed zero row (pad node)
N_NODES = 100000
G_TOT = 512
F32 = mybir.dt.float32
BF16 = mybir.dt.bfloat16
I16 = mybir.dt.int16

CH = int(os.environ.get("GCN_CH", "1024"))      # gather chunk (idx slots)
QF = int(os.environ.get("GCN_QF", "4"))         # chunks per staging tile
# NOTE: num_idxs/16 descriptors land on each of the 16 SDMA engines' SWDGE
# ring slots as m2s+s2m pairs; the ring holds 128 in-flight entries
# (_DGE_N_INFLIGHT; 2048 idxs measured to crash, 1024 verified safe), so a
# single dma_gather must keep num_idxs <= 2048 or the ring overflows and the
# device dies with NRT_EXEC_UNIT_UNRECOVERABLE.
SUP = max(CH // 16, 512) * 2                     # idx super-tile columns
GB = 8                                           # phase-1 blocks per group

LAST_RUN_INFO = {}


def _split_multiwaits(nc, max_waits=1):
    """This walrus build rejects >1 semaphore wait per instruction; hoist
    extra waits onto same-engine NOPs placed immediately before."""
    import concourse.mybir as mb
    for f in nc.m.functions:
        for blk in f.blocks:
            insts = blk.instructions
            newlist = []
            changed = False
            for inst in insts:
                si = inst.sync_info
                waits = list(si.on_wait) if si is not None and si.on_wait else []
                if len(waits) > max_waits:
                    si.on_wait = waits[-max_waits:]
                    extra = waits[:-max_waits]
                    while extra:
                        nop = mb.InstNoOp(
                            name=f"I-mwsplit-{nc.next_id()}",
                            sync_info=mb.SyncInfo(on_wait=extra[:max_waits], on_update=[]),
                            engine=inst.engine,
                            bass_nofuse=True,
                        )
                        newlist.append(nop)
                        extra = extra[max_waits:]
                    changed = True
                newlist.append(inst)
            if changed:
                insts.clear()
                insts.extend(newlist)


_COMPILED = set()


def _run(nc, in_maps, trace=False):
    if id(nc) not in _COMPILED:
        nc.compile()
        _split_multiwaits(nc)
        _COMPILED.add(id(nc))
    kw = {}
    if trace:
        kw = dict(trace=True)
    try:
        return run_bass_kernel_spmd(nc, in_maps, list(range(NC)), **kw)
    except Exception:
        # transient device-unrecoverable (wedged core from an earlier run)
        import time as _time
        _time.sleep(10)
        return run_bass_kernel_spmd(nc, in_maps, list(range(NC)), **kw)


def _pjrt_runner(nc, in_maps):
    """Build the shard_map-jitted bass_exec callable ONCE with device-resident
    inputs; returns run_once() whose wall time is dispatch + device exec only
    (fresh donated zero-outputs are re-supplied per call; for benchmarking)."""
    import jax
    import numpy as _np
    from concourse import bass2jax as b2j

    b2j.install_neuronx_cc_hook()
    partition_name = nc.partition_id_tensor.name if nc.partition_id_tensor else None
    in_names, out_names, out_avals, zero_outs = [], [], [], []
    for alloc in nc.m.functions[0].allocations:
        if not isinstance(alloc, mybir.MemoryLocationSet):
            continue
        name = alloc.memorylocations[0].name
        if alloc.kind == "ExternalInput":
            if name != partition_name:
                in_names.append(name)
        elif alloc.kind == "ExternalOutput":
            shape = tuple(alloc.tensor_shape)
            dtype = mybir.dt.np(alloc.dtype)
            out_names.append(name)
            out_avals.append(jax.core.ShapedArray(shape, dtype))
            zero_outs.append(_np.zeros(shape, dtype))
    n_params, n_outs = len(in_names), len(out_avals)
    all_in = list(in_names) + out_names + ([partition_name] if partition_name else [])

    def _body(*args):
        operands = list(args)
        if partition_name is not None:
            operands.append(b2j.partition_id_tensor())
        outs = b2j._bass_exec_p.bind(
            *operands, out_avals=tuple(out_avals), in_names=tuple(all_in),
            out_names=tuple(out_names), lowering_input_output_aliases=(),
            sim_require_finite=True, sim_require_nnan=True, nc=nc)
        return tuple(outs)

    devices = jax.devices()[:NC]
    mesh = b2j.Mesh(_np.asarray(devices), ("core",))
    donate = tuple(range(n_params, n_params + n_outs))
    sharded = jax.jit(
        b2j.shard_map(_body, mesh=mesh,
                      in_specs=(b2j.PartitionSpec("core"),) * (n_params + n_outs),
                      out_specs=(b2j.PartitionSpec("core"),) * n_outs,
                      check_rep=False),
        donate_argnums=donate, keep_unused=True)
    concat_in = [
        jax.device_put(
            _np.concatenate([_np.asarray(m[name]) for m in in_maps], axis=0))
        for name in in_names
    ]
    for a in concat_in:
        a.block_until_ready()

    def run_once():
        zs = [_np.zeros((NC * z.shape[0], *z.shape[1:]), z.dtype) for z in zero_outs]
        outs = sharded(*concat_in, *zs)
        for o in outs:
            o.block_until_ready()
        return outs

    return run_once


# ---------------------------------------------------------------- the launch


def _build_full(C, bank_chunks, merge_chunks, n_w16, loop_reps=1,
                single_packet=None):
    skip = set(os.environ.get("GCN_SKIP", "").split(","))
    if single_packet is None:
        single_packet = os.environ.get("GCN_SP", "1") == "1"
    """bank_chunks: per bank, (tail_col0, [(off16, nidx, [(gcol, zcol, ncols,
    is_round0)..])..]); merge_chunks: per bank, [(off16, nidx, col0)..];
    n_w16: total idx columns (int16 words / 16)."""
    nc = bacc.Bacc("TRN2", target_bir_lowering=False, debug=False,
                   num_swdge_queues=NQ)
    xTb = nc.declare_dram_parameter("xTb", [P, NT * P], BF16, isOutput=False)
    wgb = nc.declare_dram_parameter("wgb", [P, HID], BF16, isOutput=False)
    dinvt = nc.declare_dram_parameter("dinvt", [P, NT], F32, isOutput=False)
    idxs = nc.declare_dram_parameter("idxs", [P, n_w16], I16, isOutput=False)
    dinvz = nc.declare_dram_parameter("dinvz", [P, C], F32, isOutput=False)
    gl = nc.declare_dram_parameter("gl", [P, C], F32, isOutput=False)
    iota = nc.declare_dram_parameter("iota", [P, HID], F32, isOutput=False)
    brep = nc.declare_dram_parameter("brep", [P, HID], F32, isOutput=False)
    w1a = nc.declare_dram_parameter("w1a", [P, HID], F32, isOutput=False)
    w2a = nc.declare_dram_parameter("w2a", [P, 2], F32, isOutput=False)
    iden = nc.declare_dram_parameter("iden", [P, P], F32, isOutput=False)
    out = nc.declare_dram_parameter("out", [HID, 2], F32, isOutput=True)
    dbg = os.environ.get("GCN_DEBUG") == "1"
    if dbg:
        zdbg = nc.declare_dram_parameter("zdbg", [P, C * HID], F32, isOutput=True)
    ytab = nc.dram_tensor("ytab", [NT * P, HID], F32)
    zscr = nc.dram_tensor("zscr", [NB * P * C, HID], F32)

    with tile.TileContext(nc) as tc:
        with (
            tc.tile_pool(name="cst", bufs=1) as cst,
            tc.tile_pool(name="sb", bufs=1) as sb,
            tc.tile_pool(name="xp", bufs=3) as xp,
            tc.tile_pool(name="yp", bufs=3) as yp,
            tc.tile_pool(name="stage", bufs=3) as stage,
            tc.tile_pool(name="idxp", bufs=3) as idxp,
            tc.tile_pool(name="ps1", bufs=2, space="PSUM") as ps1,
            tc.tile_pool(name="psp", bufs=1, space="PSUM") as psp,
            tc.tile_pool(name="ps2", bufs=1, space="PSUM") as psp2,
        ):
            # ---- constants (loaded once, reused across reps)
            wg_t = cst.tile([P, HID], BF16)
            nc.scalar.dma_start(out=wg_t[:], in_=wgb[:, :])
            dit = cst.tile([P, NT, 1], F32)
            nc.scalar.dma_start(out=dit[:, :, 0], in_=dinvt[:, :])
            dz_t = cst.tile([P, C, 1], F32)
            nc.scalar.dma_start(out=dz_t[:, :, 0], in_=dinvz[:, :])
            gl_t = cst.tile([P, C, 1], F32)
            nc.scalar.dma_start(out=gl_t[:, :, 0], in_=gl[:, :])
            iota_t = cst.tile([P, 1, HID], F32)
            nc.scalar.dma_start(out=iota_t[:, 0, :], in_=iota[:, :])
            brep_t = cst.tile([P, 1, HID], F32)
            nc.scalar.dma_start(out=brep_t[:, 0, :], in_=brep[:, :])
            w1_t = cst.tile([P, HID], F32)
            nc.scalar.dma_start(out=w1_t[:], in_=w1a[:, :])
            w2_t = cst.tile([P, 2], F32)
            nc.scalar.dma_start(out=w2_t[:], in_=w2a[:, :])
            iden_t = cst.tile([P, P], F32)
            nc.scalar.dma_start(out=iden_t[:], in_=iden[:, :])
            ones_t = cst.tile([P, 1], F32)
            nc.vector.memset(ones_t[:], 1.0)
            zero_t = cst.tile([P, QF * CH // P, HID], F32)
            nc.vector.memset(zero_t[:], 0.0)

            ytab_v = ytab[:, :].rearrange("(p c) h -> p c h", p=P)

            def body():
                acc = sb.tile([P, C, HID], F32, tag="acc")
                z = sb.tile([P, C, HID], F32, tag="z")
                if "adds" in skip or "merge" in skip:
                    nc.vector.memset(acc[:], 0.0)
                    nc.vector.memset(z[:], 0.0)
                sup_state = {"s0": -1, "tile": None}

                def get_idx(off16, w):
                    if (sup_state["s0"] < 0 or off16 < sup_state["s0"]
                            or off16 + w > sup_state["s0"] + SUP):
                        w2 = min(SUP, n_w16 - off16)
                        t = idxp.tile([P, SUP], I16, tag="idx")
                        nc.scalar.dma_start(out=t[:, :w2], in_=idxs[:, off16:off16 + w2])
                        sup_state["s0"] = off16
                        sup_state["tile"] = t
                    o = off16 - sup_state["s0"]
                    return sup_state["tile"][:, o:o + w]

                gq = [0]

                def gather(dst_ap, src_ap, off16, nidx):
                    it = get_idx(off16, nidx // 16)
                    gi = nc.gpsimd.dma_gather(dst_ap, src_ap, it, nidx, nidx,
                                              HID, queue_num=gq[0] % NQ,
                                              single_packet=single_packet)
                    gq[0] += 1
                    return gi

                # ---------------- phase 1: y table (all nodes, rank order)
                ywrites = []
                for g in range(0 if "p1" in skip else NT // GB):
                    xt = xp.tile([P, GB * P], BF16, tag="xt")
                    nc.scalar.dma_start(
                        out=xt[:], in_=xTb[:, g * GB * P:(g + 1) * GB * P])
                    psb = ps1.tile([P, GB, HID], F32, space="PSUM", tag="ps")
                    for k in range(GB):
                        nc.tensor.matmul(out=psb[:, k, :],
                                         lhsT=xt[:, k * P:(k + 1) * P],
                                         rhs=wg_t[:], start=True, stop=True)
                    y8 = yp.tile([P, GB, HID], F32, tag="y8")
                    nc.vector.tensor_tensor(
                        out=y8[:], in0=psb[:],
                        in1=dit[:, g * GB:(g + 1) * GB, :]
                            .broadcast_to([P, GB, HID]),
                        op=mybir.AluOpType.mult)
                    wi = nc.scalar.dma_start(
                        out=ytab_v[:, g * GB:(g + 1) * GB, :], in_=y8[:])
                    ywrites.append(wi)

                # ---------------- phase 2 + interleaved phase 3:
                # bank b's merge gathers are emitted after bank b+1's main
                # gathers so the dump DMA completes off the Pool critical
                # path and merge work hides under later banks.
                first_on_queue = set()
                dump_insts = []

                def merge_bank(b):
                    for (subs, col0) in merge_chunks[b]:
                        st = stage.tile([P, QF * CH // P, HID], F32, tag="st")
                        qcols = sum(nidx for (_, nidx, _) in subs) // P
                        if "gather" not in skip:
                            for (off16, nidx, scol) in subs:
                                gi = gather(st[:, scol: scol + nidx // P, :],
                                            zscr[b * P * C:(b + 1) * P * C, :],
                                            off16, nidx)
                                tile.add_dep_helper(gi.ins, dump_insts[b].ins,
                                                    sync=True,
                                                    reason="merge gather reads zscr dump")
                        if "adds" in skip:
                            continue
                        if b == 0:
                            nc.vector.tensor_tensor(
                                out=z[:, col0:col0 + qcols, :],
                                in0=st[:, : qcols, :],
                                in1=zero_t[:, :qcols, :],
                                op=mybir.AluOpType.add)
                        else:
                            nc.vector.tensor_tensor(
                                out=z[:, col0:col0 + qcols, :],
                                in0=z[:, col0:col0 + qcols, :],
                                in1=st[:, : qcols, :],
                                op=mybir.AluOpType.add)

                for b in range(NB):
                    tail_col0, chunks = bank_chunks[b]
                    if tail_col0 < C:
                        nc.vector.memset(acc[:, tail_col0:, :], 0.0)
                    for (subs, pieces) in chunks:
                        st = stage.tile([P, QF * CH // P, HID], F32, tag="st")
                        if "gather" not in skip:
                            for (off16, nidx, scol) in subs:
                                qn = gq[0] % NQ
                                gi = gather(st[:, scol: scol + nidx // P, :],
                                            ytab[b * VB:(b + 1) * VB, :],
                                            off16, nidx)
                                if qn not in first_on_queue and ywrites:
                                    first_on_queue.add(qn)
                                    tile.add_dep_helper(gi.ins, ywrites[-1].ins,
                                                        sync=True,
                                                        reason="gather reads ytab")
                        if "adds" in skip:
                            continue
                        for (gcol, zcol, ncols, r0) in pieces:
                            if r0:
                                nc.vector.tensor_tensor(
                                    out=acc[:, zcol:zcol + ncols, :],
                                    in0=st[:, gcol:gcol + ncols, :],
                                    in1=zero_t[:, :ncols, :],
                                    op=mybir.AluOpType.add)
                            else:
                                nc.vector.tensor_tensor(
                                    out=acc[:, zcol:zcol + ncols, :],
                                    in0=acc[:, zcol:zcol + ncols, :],
                                    in1=st[:, gcol:gcol + ncols, :],
                                    op=mybir.AluOpType.add)
                    if "merge" not in skip:
                        di = nc.scalar.dma_start(
                            out=zscr[b * P * C:(b + 1) * P * C, :],
                            in_=acc[:].rearrange("p c h -> p (c h)"))
                        dump_insts.append(di)
                        if b > 0:
                            merge_bank(b - 1)

                # ---------------- phase 3 tail: last bank's merge
                if "merge" not in skip:
                    merge_bank(NB - 1)

                # ---------------- phase 4: scale+bias+relu, pool, MLP
                nc.vector.tensor_tensor(
                    out=z[:], in0=z[:],
                    in1=dz_t[:].broadcast_to([P, C, HID]),
                    op=mybir.AluOpType.mult)
                nc.vector.tensor_tensor(
                    out=z[:], in0=z[:],
                    in1=brep_t[:].broadcast_to([P, C, HID]),
                    op=mybir.AluOpType.add)
                zf = z[:].rearrange("p c h -> p (c h)")
                nc.scalar.activation(zf, zf, mybir.ActivationFunctionType.Relu)
                if dbg:
                    nc.scalar.dma_start(out=zdbg[:, :], in_=zf)
                # pooling: one-hot PSUM matmuls
                oh = sb.tile([P, C, HID], F32, tag="oh")
                nc.vector.tensor_tensor(
                    out=oh[:],
                    in0=gl_t[:].broadcast_to([P, C, HID]),
                    in1=iota_t[:].broadcast_to([P, C, HID]),
                    op=mybir.AluOpType.is_equal)
                ps_sum = psp.tile([HID, HID], F32, space="PSUM", tag="pssum")
                ps_cnt = psp.tile([HID, 1], F32, space="PSUM", tag="pscnt")
                for c in range(C):
                    nc.tensor.matmul(out=ps_sum[:], lhsT=oh[:, c, :], rhs=z[:, c, :],
                                     start=(c == 0), stop=(c == C - 1),
                                     skip_group_check=True)
                    nc.tensor.matmul(out=ps_cnt[:], lhsT=oh[:, c, :], rhs=ones_t[:],
                                     start=(c == 0), stop=(c == C - 1),
                                     skip_group_check=True)
                cnt = sb.tile([HID, 1], F32, tag="cnt")
                nc.vector.tensor_scalar_max(cnt[:], ps_cnt[:], 1.0)
                nc.vector.reciprocal(cnt[:], cnt[:])
                g_sb = sb.tile([HID, HID], F32, tag="gsb")
                nc.vector.tensor_tensor(out=g_sb[:], in0=ps_sum[:],
                                        in1=cnt[:].broadcast_to([HID, HID]),
                                        op=mybir.AluOpType.mult)
                # MLP with homogeneous-coordinate bias
                gT = psp2.tile([HID, HID], F32, space="PSUM", tag="tr")
                nc.tensor.transpose(out=gT[:], in_=g_sb[:], identity=iden_t[:HID, :HID])
                a1 = sb.tile([P, HID], F32, tag="a1")
                nc.vector.memset(a1[HID:HID + 1, :], 1.0)
                nc.vector.tensor_copy(a1[:HID, :], gT[:])
                h_ps = psp2.tile([HID, HID], F32, space="PSUM", tag="mm")
                nc.tensor.matmul(out=h_ps[:], lhsT=a1[0:HID + 1, :], rhs=w1_t[0:HID + 1, :],
                                 start=True, stop=True)
                h_sb = sb.tile([HID, HID], F32, tag="hsb")
                nc.scalar.activation(h_sb[:], h_ps[:], mybir.ActivationFunctionType.Relu)
                hT = psp2.tile([HID, HID], F32, space="PSUM", tag="tr2")
                nc.tensor.transpose(out=hT[:], in_=h_sb[:], identity=iden_t[:HID, :HID])
                a2 = sb.tile([P, HID], F32, tag="a2")
                nc.vector.memset(a2[HID:HID + 1, :], 1.0)
                nc.vector.tensor_copy(a2[:HID, :], hT[:])
                o_ps = psp2.tile([HID, 2], F32, space="PSUM", tag="mm2")
                nc.tensor.matmul(out=o_ps[:], lhsT=a2[0:HID + 1, :], rhs=w2_t[0:HID + 1, :],
                                 start=True, stop=True)
                o_sb = sb.tile([HID, 2], F32, tag="osb")
                nc.scalar.activation(o_sb[:], o_ps[:], mybir.ActivationFunctionType.Sigmoid)
                nc.scalar.dma_start(out=out[:, :], in_=o_sb[:])

            if loop_reps > 1:
                with tc.For_i(0, loop_reps, 1):
                    body()
            else:
                for _ in range(int(os.environ.get("GCN_REPS", "1"))):
                    body()
    return nc


# ---------------------------------------------------------------- host glue


def _wrap16(vals):
    """int16 stream -> [128, ceil(n/16)] ucode layout (16-wrapped, 8x repl)."""
    n = len(vals)
    w = (n + 15) // 16
    a = np.full(w * 16, -1, np.int16)
    a[:n] = vals
    blk = a.reshape(w, 16).T
    return np.tile(blk, (8, 1))


def _prep(x, edge_index, batch, W_gcn, b_gcn, W1, b1, W2, b2):
    """All host-side scheduling; returns (build_args, in_maps, meta)."""
    import ml_dtypes
    x = np.ascontiguousarray(np.asarray(x, dtype=np.float32))
    ei = np.asarray(edge_index).astype(np.int64)
    batch_np = np.asarray(batch).astype(np.int64)
    W_gcn = np.asarray(W_gcn, np.float32); b_gcn = np.asarray(b_gcn, np.float32)
    W1 = np.asarray(W1, np.float32); b1 = np.asarray(b1, np.float32)
    W2 = np.asarray(W2, np.float32); b2 = np.asarray(b2, np.float32)

    N = x.shape[0]
    G = G_TOT
    row = ei[0]
    col = ei[1]
    sl = np.arange(N, dtype=np.int64)
    row2 = np.concatenate([row, sl])
    col2 = np.concatenate([col, sl])
    deg = np.bincount(col2, minlength=N).astype(np.float32)  # >=1 always
    dinv = 1.0 / np.sqrt(deg)

    gpc = G // NC
    gb = np.searchsorted(batch_np, np.arange(0, G + 1, gpc))
    Ncs = np.diff(gb)
    C = int((Ncs.max() + P - 1) // P)

    # source-node -> (bank, int16 bank-local rank)
    sp = row2 % P
    sc = row2 // P
    sbank = (sp // 32).astype(np.int64)
    sidx = ((sp % 32) * NT + sc).astype(np.int64)   # < 25088

    # phase-1 host tensors (shared across cores)
    xTb = np.zeros((P, NT * P), ml_dtypes.bfloat16)
    xTb[:, :N] = x.T.astype(ml_dtypes.bfloat16)
    wgbf = W_gcn.astype(ml_dtypes.bfloat16)
    dinvt = np.zeros((P, NT), np.float32)
    dflat = dinvt.reshape(-1, order="F")            # (p,c) -> c*128+p
    dflat[:N] = dinv
    dinvt = dflat.reshape(NT, P).T.copy()

    # per-core edge sets
    core_data = []
    for c in range(NC):
        lo, hi = int(gb[c]), int(gb[c + 1])
        m = (col2 >= lo) & (col2 < hi)
        core_data.append((lo, hi, sidx[m], (col2[m] - lo), sbank[m]))

    # common round schedule per bank
    bank_rounds = []
    for b in range(NB):
        per_core = []
        for c in range(NC):
            lo, hi, r_c, cl, bank = core_data[c]
            nloc = hi - lo
            degb = np.bincount(cl[bank == b], minlength=nloc)
            if degb.max() == 0:
                per_core.append(np.zeros(0, np.int64))
                continue
            h = np.bincount(degb)
            nbr = (nloc - np.cumsum(h))[:len(h) - 1]
            per_core.append(np.asarray(nbr, np.int64))
        R = max((len(a) for a in per_core), default=0)
        Nbr = np.zeros(R, np.int64)
        for a in per_core:
            aa = np.zeros(R, np.int64)
            aa[:len(a)] = a
            Nbr = np.maximum(Nbr, ((aa + P - 1) // P) * P)
        bank_rounds.append(Nbr)

    # quad schedule (common): per bank, quads of <=4*CH slots; each quad is
    # up to 4 dma_gather subchunks into one staging tile + round pieces
    # covering the whole quad range.
    QCH = QF * CH
    bank_chunks = []
    bank_off16 = []
    off16 = 0
    for b in range(NB):
        Nbr = bank_rounds[b]
        S = int(Nbr.sum())
        starts = np.concatenate([[0], np.cumsum(Nbr)])
        quads = []
        pos = 0
        while pos < S:
            qln = min(QCH, S - pos)
            subs = []
            sp = pos
            while sp < pos + qln:
                ln = min(CH, pos + qln - sp)
                subs.append((off16 + sp // 16, int(ln), int((sp - pos) // P)))
                sp += ln
            pieces = []
            for r in range(len(Nbr)):
                a = max(pos, starts[r]); e = min(pos + qln, starts[r + 1])
                if a < e:
                    pieces.append((int((a - pos) // P), int((a - starts[r]) // P),
                                   int((e - a) // P), bool(r == 0)))
            quads.append((subs, pieces))
            pos += qln
        tail_col0 = int(Nbr[0] // P) if len(Nbr) else 0
        bank_chunks.append((tail_col0, quads))
        bank_off16.append(off16)
        off16 += S // 16
    # merge quads (common): C*128 idxs per bank
    merge_chunks = []
    merge_off16 = []
    for b in range(NB):
        Sm = C * P
        quads = []
        pos = 0
        while pos < Sm:
            qln = min(QCH, Sm - pos)
            subs = []
            sp = pos
            while sp < pos + qln:
                ln = min(CH, pos + qln - sp)
                subs.append((off16 + sp // 16, int(ln), int((sp - pos) // P)))
                sp += ln
            quads.append((subs, int(pos // P)))
            pos += qln
        merge_chunks.append(quads)
        merge_off16.append(off16)
        off16 += Sm // 16
    n_w16 = off16

    # per-core idx streams + small tensors
    in_maps = []
    iota64 = np.tile(np.arange(HID, dtype=np.float32), (P, 1))
    brep = np.tile(b_gcn[None, :], (P, 1)).astype(np.float32)
    w1a = np.zeros((P, HID), np.float32); w1a[:HID] = W1; w1a[HID] = b1
    w2a = np.zeros((P, 2), np.float32); w2a[:HID] = W2; w2a[HID] = b2
    iden = np.eye(P, dtype=np.float32)
    for c in range(NC):
        lo, hi, r_c, cl, bank = core_data[c]
        nloc = hi - lo
        idxbuf = np.empty(n_w16 * 16, np.int16)
        for b in range(NB):
            Nbr = bank_rounds[b]
            S = int(Nbr.sum())
            starts = np.concatenate([[0], np.cumsum(Nbr)])
            stream = np.full(S, DUMMY, np.int16)
            mb = bank == b
            rb, clb = r_c[mb], cl[mb]
            degb = np.bincount(clb, minlength=nloc)
            order = np.argsort(-degb, kind="stable")   # bank-rank -> node
            rank = np.empty(nloc, np.int64)
            rank[order] = np.arange(nloc)
            rk = rank[clb]
            o = np.lexsort((np.arange(len(rk)), rk))
            rk_s, src_s = rk[o], rb[o]
            grp_start = np.searchsorted(rk_s, rk_s)
            j = np.arange(len(rk_s)) - grp_start
            stream[starts[j] + rk_s] = src_s.astype(np.int16)
            idxbuf[bank_off16[b] * 16: bank_off16[b] * 16 + S] = stream
            # merge idx for this bank: node order -> acc_b row
            mrow = (rank % P) * C + (rank // P)
            mstream = np.zeros(C * P, np.int16)
            mstream[:nloc] = mrow.astype(np.int16)
            idxbuf[merge_off16[b] * 16: merge_off16[b] * 16 + C * P] = mstream
        idxw = _wrap16(idxbuf)
        dz = np.zeros(C * P, np.float32); dz[:nloc] = dinv[lo:hi]
        glv = np.full(C * P, float(HID), np.float32)
        glv[:nloc] = (batch_np[lo:hi] - c * gpc).astype(np.float32)
        in_maps.append({
            "xTb": xTb, "wgb": wgbf, "dinvt": dinvt, "idxs": idxw,
            "dinvz": dz.reshape(C, P).T.copy(),
            "gl": glv.reshape(C, P).T.copy(),
            "iota": iota64, "brep": brep, "w1a": w1a, "w2a": w2a, "iden": iden,
        })

    build_args = (C, bank_chunks, merge_chunks, n_w16)
    return build_args, in_maps, dict(gb=gb, C=C)


_NC_CACHE = {}


def kernel(x, edge_index, batch, W_gcn, b_gcn, W1, b1, W2, b2):
    build_args, in_maps, meta = _prep(x, edge_index, batch, W_gcn, b_gcn,
                                      W1, b1, W2, b2)
    LAST_RUN_INFO["build_args"] = build_args
    LAST_RUN_INFO["in_maps"] = in_maps
    LAST_RUN_INFO.update(meta)
    key = repr(build_args)
    if key not in _NC_CACHE:
        _NC_CACHE[key] = _build_full(*build_args)
    nc = _NC_CACHE[key]
    trace = os.environ.get("GCN_TRACE") == "1"
    r = _run(nc, in_maps, trace=trace)
    LAST_RUN_INFO["exec_ns"] = r.exec_time_ns
    if os.environ.get("GCN_DEBUG") == "1":
        LAST_RUN_INFO["zdbg"] = [r.results[c]["zdbg"].reshape(P, -1, HID)
                                 for c in range(NC)]
    out = np.concatenate([r.results[c]["out"] for c in range(NC)], axis=0)
    return out[:G_TOT].astype(np.float32)
